# revision 2
# baseline (speedup 1.0000x reference)
"""Multi-head attention (B=4, S=2048, H=1024, NH=16) on 8 TRN2 NeuronCores.

Sharding: data-parallel over batch (4) x tensor-parallel over heads (2 groups
of 8 heads). Core c handles batch c//2, head-group c%2 (features 512*(c%2)..).

Per-core kernel (cost-model-optimized):
  1. Projections in fp8e4 hi/lo pairs with DoubleRow matmuls: the host splits
     x^T and 32*W^T into fp8 (hi, lo) pairs; x@W.T is computed as
     xh@Wh + xh@Wl + xl@Wh (the dropped xl@Wl term is ~2^-16 relative),
     accumulating 3 DoubleRow groups of 4 chunk-pair matmuls in PSUM.
     Accuracy is bf16-level or better; modeled PE cost is 4x below bf16.
     Q^T/K^T land in SBUF as float32r [feature, token]; V as bf16
     [token, feature] with a ones column per (pair, head, kt-chunk) that
     accumulates the softmax denominator during PV.
  2. Attention per (pair, q-block of 512) in four 4-chunk panels:
       - two row-tiled QK^T matmuls (f32r) produce S^T [128 kt, 2x512 q],
       - exp(s/8192 + mask - 4): 13/16 chunks on ScalarE (true exp -> bf16),
         3/16 on VectorE via a Schraudolph bit-trick (one tensor_scalar:
         i16 = round(A*s + B); the int16 bits ARE bf16 exp to +-3 percent,
         harmless for softmax weights at this fraction),
       - transposed PV: stationary = P^T [128 kt, 128 q], moving =
         V|ones [128 kt, 65] -> ctx accumulates as [q, feature] in PSUM,
         halving modeled PE time and giving [token, feature] output with
         no host transpose. Panel partials accumulate in SBUF so a panel's
         PSUM tile recycles quickly (PSUM is the scarcest resource).
  3. Emission is hand-woven: projection rounds are interleaved into the
     attention chunk stream at the cadence the engines actually consume
     them, sharing one 3-deep PSUM ring, so ScalarE (the critical engine)
     never starves while the PE works through projections.
  4. ctx [128, 4*65] per (pair, head, qb) -> SBUF -> DRAM; the host divides
     by the sumexp column and the 32x weight scale.
"""

import math
import os
from contextlib import ExitStack

import numpy as np

import concourse.mybir as mybir
import concourse.tile as tile
from concourse import bacc
from concourse.bass_utils import run_bass_kernel_spmd

B, S, H, NH, HD = 4, 2048, 1024, 16, 64
NCORES = 8
DP, TP = 4, 2            # batch-parallel x head-group-parallel
HG = NH // TP            # 8 heads per core
DG = HG * HD             # 512 features per core
NPAIR = HG // 2          # 4 head pairs (128 features each)
CCH = H // 128           # 8 contraction chunks of 128
CCP = CCH // 2           # 4 chunk-pairs of 256 (DoubleRow)
TB = S // 512            # 4 token blocks of 512
TCH = S // 128           # 16 kt chunks of 128
QB = S // 512            # 4 q-blocks of 512
F32 = mybir.dt.float32
F32R = mybir.dt.float32r
BF16 = mybir.dt.bfloat16
I16 = mybir.dt.int16
FP8 = mybir.dt.float8e4
Alu = mybir.AluOpType
DR = mybir.MatmulPerfMode.DoubleRow

WSCALE = 32.0                   # host multiplies W^T by this before hi/lo split
SSCALE = WSCALE * WSCALE        # score scale (q and k each carry WSCALE)
C0 = 4.0                        # score shift: exp(s - C0); cancels in softmax
SA = 1.0 / (8.0 * SSCALE)       # activation scale: exp(SA*s' + mask - C0)
LOG2E = math.log2(math.e)
SCHRAU_A = 128.0 * LOG2E * SA   # DVE: i16 = round(SCHRAU_A*s' + sbias)
SCHRAU_B = 16256.0 - 128.0 * 0.0430
DVE_CHUNKS = (3, 7, 11, 15)     # kt-chunks whose exp runs on VectorE

_CACHED = None        # compiled Bass program (left bare for test harnesses)
_CACHED_KEY = None
LAST_RESULTS = None   # BassKernelResults of the most recent run (for test.py)
TRACE = False


def _build_core_program(has_bv=True):
    nc = bacc.Bacc(
        "TRN2", target_bir_lowering=False, debug=False, enable_asserts=False
    )

    xs = {}
    for nm in ("xq", "xk", "xv"):
        for part in ("h", "l"):
            xs[nm + part] = nc.declare_dram_parameter(
                nm + part, [H, S], FP8, isOutput=False
            )
    ws = {}
    for nm in ("wq", "wk", "wv"):
        for part in ("h", "l"):
            ws[nm + part] = nc.declare_dram_parameter(
                nm + part, [H, DG], FP8, isOutput=False
            )
    bq = nc.declare_dram_parameter("bq", [128, NPAIR], F32, isOutput=False)
    bk = nc.declare_dram_parameter("bk", [128, NPAIR], F32, isOutput=False)
    bv = nc.declare_dram_parameter("bv", [1, DG], BF16, isOutput=False)
    embias = nc.declare_dram_parameter("embias", [128, TCH], F32, isOutput=False)
    sbias = nc.declare_dram_parameter("sbias", [128, TCH], F32, isOutput=False)
    out = nc.declare_dram_parameter("out", [S, HG * 65], F32, isOutput=True)
    dbg = None
    if os.environ.get("KDEBUG") == "1":
        dbg = nc.declare_dram_parameter("dbg", [128, 2 * S], F32, isOutput=True)

    with tile.TileContext(nc) as tc:
        _emit(tc, nc, xs, ws, bq, bk, bv, embias, sbias, out, has_bv, dbg)

    nc.compile()
    return nc


def _emit(tc, nc, xs, ws, bq, bk, bv, embias, sbias, out, HAS_BV, dbg=None):
    Exp = mybir.ActivationFunctionType.Exp

    pools = ExitStack()
    const = pools.enter_context(tc.tile_pool(name="const", bufs=1))
    persist = pools.enter_context(tc.tile_pool(name="persist", bufs=1))
    xpool = pools.enter_context(tc.tile_pool(name="xpool", bufs=3))
    ptpool = pools.enter_context(tc.tile_pool(name="ptpool", bufs=6))
    work = pools.enter_context(tc.tile_pool(name="work", bufs=3))
    psum = pools.enter_context(tc.tile_pool(name="psum", bufs=1, space="PSUM"))

    # ---- constants ----
    ones_row = const.tile([1, 512], BF16, tag="ones_row")
    nc.gpsimd.memset(ones_row[:], 1.0)

    bq_sb = const.tile([128, NPAIR], F32, tag="bq")
    bk_sb = const.tile([128, NPAIR], F32, tag="bk")
    bv_sb = const.tile([1, DG], BF16, tag="bv")
    em_sb = const.tile([128, TCH], F32, tag="embias")
    sb_sb = const.tile([128, TCH], F32, tag="sbias")

    def load_consts_critical():
        nc.sync.dma_start(bk_sb[:], bk[:])
        nc.sync.dma_start(bq_sb[:], bq[:])
        nc.sync.dma_start(em_sb[:], embias[:])

    def load_consts_rest():
        nc.sync.dma_start(sb_sb[:], sbias[:])
        nc.sync.dma_start(bv_sb[:], bv[:])

    # ---- weights: [128, cch*DG] fp8, chunk-major; one DMA per tensor ----
    w_sb = {}

    def load_w(key):
        if key in w_sb:
            return w_sb[key]
        w = const.tile([128, CCH * DG], FP8, tag=f"w{key}", name=f"w{key}")
        nc.sync.dma_start(
            w[:].rearrange("p (c m) -> p c m", c=CCH),
            ws[key][:, :].rearrange("(c p) m -> p c m", p=128),
        )
        w_sb[key] = w
        return w

    # ---- persistent activations ----
    qt_sb = [
        persist.tile([128, S], F32R, tag=f"qt{p}", name=f"qt{p}")
        for p in range(NPAIR)
    ]
    kt_sb = [
        persist.tile([128, S], F32R, tag=f"kt{p}", name=f"kt{p}")
        for p in range(NPAIR)
    ]
    # V with ones column per (pair, kt-chunk, head): [t, p, 2c+h, 65]
    v_sb = persist.tile([128, NPAIR * TCH * 2 * 65], BF16, tag="v")
    v_r = v_sb[:].rearrange("t (p c2 d) -> t p c2 d", p=NPAIR, c2=TCH * 2)
    nc.gpsimd.memset(v_r[:, :, :, 64:65], 1.0)

    # SBUF partial-ctx accumulators, one per (pair, head, qb)
    csp = {
        (pr, h, qb): persist.tile(
            [128, 260], F32, tag=f"csp{pr}_{h}_{qb}", name=f"csp{pr}_{h}_{qb}"
        )
        for pr in range(NPAIR)
        for h in (0, 1)
        for qb in range(QB)
    }

    # ---- x streaming: one DMA per (proj, half, tb) ----
    xt_sb = {}

    def do_prefetch(r, names=("xk", "xq", "xv")):
        for nm in names:
            for part in ("h", "l"):
                t = xpool.tile(
                    [128, CCH * 512], FP8, tag=f"x{nm}{part}", bufs=2,
                    name=f"x{nm}{part}{r}",
                )
                nc.sync.dma_start(
                    t[:].rearrange("p (c t) -> p c t", c=CCH),
                    xs[nm + part][:, 512 * r : 512 * (r + 1)]
                    .rearrange("(c p) t -> p c t", p=128),
                )
                xt_sb[(nm, part, r)] = t

    # ---- projection groups, split into 3 parts of 4 matmuls each so the
    # in-order PE stream never runs more than ~430ns of projection work
    # between attention chunks (ScalarE stays saturated).
    def qk_group_parts(wkey, nm, dst, bias_sb, pr, tb):
        state = {}
        terms = [(f"{wkey}h", 0), (f"{wkey}l", 0), (f"{wkey}h", 1)]

        def part(k):
            def go():
                xh = xt_sb[(nm, "h", tb)]
                xl = xt_sb[(nm, "l", tb)]
                xts = (xh, xl)
                if k == 0:
                    state["pst"] = psum.tile(
                        [128, 1024], F32, tag="mm", bufs=3, name="ps"
                    )
                ps = state["pst"][:, 0:512]
                for i in range(4 * k, 4 * k + 4):
                    wk, xi = terms[i // CCP]
                    ccp = i % CCP
                    rhs = xts[xi][
                        :, (2 * ccp) * 512 : (2 * ccp + 2) * 512
                    ].rearrange("a (two n) -> a two n", two=2)
                    lhsT = (
                        w_sb[wk][:, (2 * ccp) * DG : (2 * ccp + 2) * DG]
                        .rearrange("a (two m) -> a two m", two=2)[
                            :, :, 128 * pr : 128 * (pr + 1)
                        ]
                    )
                    nc.tensor.matmul(
                        ps, lhsT, rhs, start=(i == 0), stop=(i == 11),
                        perf_mode=DR,
                    )
                if k == 2:
                    nc.vector.tensor_scalar_add(
                        dst[pr][:, 512 * tb : 512 * (tb + 1)],
                        ps,
                        bias_sb[:, pr : pr + 1],
                    )
            return go

        return [part(0), part(1), part(2)]

    def v_group_parts(tb, j):
        state = {}
        c = 4 * tb + j
        terms = [("wvh", 0), ("wvh", 1), ("wvl", 0)]

        def part(k):
            def go():
                xh = xt_sb[("xv", "h", tb)]
                xl = xt_sb[("xv", "l", tb)]
                xts = (xh, xl)
                if k == 0:
                    state["pst"] = psum.tile(
                        [128, 1024], F32, tag="mm", bufs=3, name="ps"
                    )
                ps = state["pst"][:, 0:512]
                for i in range(4 * k, 4 * k + 4):
                    wk, xi = terms[i // CCP]
                    ccp = i % CCP
                    lhsT = (
                        xts[xi][:, (2 * ccp) * 512 : (2 * ccp + 2) * 512]
                        .rearrange("a (two t) -> a two t", two=2)[
                            :, :, 128 * j : 128 * (j + 1)
                        ]
                    )
                    rhs = (
                        w_sb[wk][:, (2 * ccp) * DG : (2 * ccp + 2) * DG]
                        .rearrange("a (two m) -> a two m", two=2)
                    )
                    nc.tensor.matmul(
                        ps, lhsT, rhs,
                        start=(i == 0),
                        stop=(i == 11 and not HAS_BV),
                        perf_mode=DR,
                    )
                if k == 2:
                    if HAS_BV:
                        nc.tensor.matmul(
                            ps, ones_row[:1, 0:128], bv_sb[:1, :],
                            start=False, stop=True,
                        )
                    nc.vector.tensor_copy(
                        out=v_r[:, :, 2 * c : 2 * c + 2, 0:64],
                        in_=ps.rearrange("t (p h d) -> t p h d", p=NPAIR, h=2),
                    )
            return go

        return [part(0), part(1), part(2)]

    def round_parts(r, kq_first=False):
        parts = []
        if kq_first:
            for pr in range(NPAIR):
                parts.extend(qk_group_parts("wk", "xk", kt_sb, bk_sb, pr, r))
                parts.extend(qk_group_parts("wq", "xq", qt_sb, bq_sb, pr, r))
            for j in range(4):
                parts.extend(v_group_parts(r, j))
            return parts
        for pr in range(NPAIR):
            parts.extend(qk_group_parts("wk", "xk", kt_sb, bk_sb, pr, r))
        for j in range(4):
            parts.extend(v_group_parts(r, j))
        for pr in range(NPAIR):
            parts.extend(qk_group_parts("wq", "xq", qt_sb, bq_sb, pr, r))
        return parts

    # ---- attention pieces ----
    ctx_of = {}
    pt_of = {}
    _defer_state = {"list": None, "slot": 0}

    def emit_deferred(thunk):
        _defer_state["list"].append((_defer_state["slot"] + 2, thunk))

    def emit_qk_exp(pr, qb, c):
        q_sl = slice(512 * qb, 512 * (qb + 1))
        kt_sl = slice(128 * c, 128 * (c + 1))
        sc = psum.tile([128, 1024], F32, tag="mm", bufs=3)
        for h in (0, 1):
            hp = slice(64 * h, 64 * (h + 1))
            nc.tensor.matmul(
                sc[:, 512 * h : 512 * (h + 1)],
                kt_sb[pr][hp, kt_sl],
                qt_sb[pr][hp, q_sl],
                start=True, stop=True,
            )
        pt = ptpool.tile([128, 1024], I16, tag="pt", bufs=6)
        if c in DVE_CHUNKS:
            nc.vector.tensor_scalar(
                out=pt[:], in0=sc[:],
                scalar1=float(SCHRAU_A), scalar2=sb_sb[:, c : c + 1],
                op0=Alu.mult, op1=Alu.add,
            )
        else:
            nc.scalar.activation(
                pt[:].bitcast(BF16), sc[:], Exp,
                bias=em_sb[:, c : c + 1], scale=float(SA),
            )
        pt_of[(pr, qb, c)] = pt

    def emit_pv(pr, qb, c):
        # 4-chunk ctx panels; partials accumulate into csp in SBUF
        ptb = pt_of.pop((pr, qb, c))[:].bitcast(BF16)
        for h in (0, 1):
            if c % 4 == 0:
                ctx_of[(pr, h, qb)] = psum.tile(
                    [128, 260], F32, tag="ctx", bufs=2,
                    name=f"ctx{pr}_{h}_{qb}_{c // 4}",
                )
            ctx_ps = ctx_of[(pr, h, qb)]
            for j in range(4):
                nc.tensor.matmul(
                    ctx_ps[:, 65 * j : 65 * j + 65],
                    ptb[:, 512 * h + 128 * j : 512 * h + 128 * (j + 1)],
                    v_r[:, pr, 2 * c + h, :],
                    start=(c % 4 == 0 and j == 0),
                    stop=(c % 4 == 3 and j == 3),
                )
            if c % 4 == 3:
                ctx_done = ctx_of.pop((pr, h, qb))
                acc = csp[(pr, h, qb)]

                def flush(pr=pr, h=h, qb=qb, c=c, ctx_done=ctx_done, acc=acc):
                    if c == 3:
                        nc.vector.tensor_copy(out=acc[:], in_=ctx_done[:])
                    elif c < TCH - 1:
                        nc.vector.tensor_add(acc[:], acc[:], ctx_done[:])
                    else:
                        gh = 2 * pr + h
                        cs = work.tile(
                            [128, 260], F32, tag="cs", bufs=4, name="cs"
                        )
                        nc.vector.tensor_add(cs[:], acc[:], ctx_done[:])
                        nc.sync.dma_start(
                            out[
                                512 * qb : 512 * (qb + 1),
                                65 * gh : 65 * (gh + 1),
                            ].rearrange("(j q) d -> q j d", j=4),
                            cs[:].rearrange("q (j d) -> q j d", j=4),
                        )

                emit_deferred(flush)

    # ---- the weave ----
    # round 0 first (attention panel-group 0 needs K0/Q0/V0), then
    # panel-groups g=0..3 (chunks 4g..4g+3 of every block), with round
    # g+1's projection groups spread through panel-group g's slots.
    load_consts_critical()
    load_w("wkh"); load_w("wkl")
    do_prefetch(0, names=("xk",))
    load_w("wqh"); load_w("wql")
    do_prefetch(0, names=("xq",))
    load_w("wvh"); load_w("wvl")
    do_prefetch(0, names=("xv",))
    load_consts_rest()
    r0 = round_parts(0, kq_first=True)   # [K0p,Q0p]x4 pairs then V0 j0..3

    # Wavefront: slot window W_g holds the block-panels (pr, qb, panel) with
    # max(qb, panel) == g -- exactly what rounds 0..g have produced. Round
    # g+1's projection parts are woven through W_g's slots.
    flat = []
    wstart = []
    for g in range(4):
        wstart.append(len(flat))
        for qb in range(QB):
            for p in range(4):
                if max(qb, p) != g:
                    continue
                for pr in range(NPAIR):
                    for cc in range(4):
                        flat.append((pr, qb, 4 * p + cc))

    # part insertion schedule: slot -> list of part thunks.
    # round 0: [K0p Q0p V0j] trios before pair p's first chunks of W0.
    ins = {}
    # K0/Q0 per pair p must land before slot 4p (pair p's first QK);
    # V0 j-group must land >= 2 slots before the PV that reads chunk j
    # (PV for chunk c trails its slot by the skew).
    ins.setdefault(0, []).extend(r0[0:6] + r0[24:27])        # K0p0 Q0p0 V0j0
    ins.setdefault(1, []).extend(r0[27:30] + r0[6:9])        # V0j1 K0p1
    ins.setdefault(2, []).extend(r0[30:33] + r0[9:12])       # V0j2 Q0p1
    ins.setdefault(3, []).extend(r0[33:36] + r0[12:15])      # V0j3 K0p2
    ins.setdefault(4, []).extend(r0[15:18])                  # Q0p2
    ins.setdefault(5, []).extend(r0[18:24])                  # K0p3 Q0p3
    for r in range(1, TB):
        parts = round_parts(r)                 # 36 parts
        base, span = wstart[r - 1] + 1, (wstart[r] - wstart[r - 1]) - 2
        for idx, t in enumerate(parts):
            ins.setdefault(base + (idx * span) // 36, []).append(t)

    prefetch_at = {1: 1, wstart[1]: 2, wstart[2]: 3}
    skew = 2
    deferred = []   # (due_slot, thunk) for csp flush-adds
    _defer_state["list"] = deferred
    for i, cur in enumerate(flat):
        _defer_state["slot"] = i
        if i in prefetch_at:
            do_prefetch(prefetch_at[i])
        for t in ins.get(i, ()):
            t()
        emit_qk_exp(*cur)
        if i >= skew:
            emit_pv(*flat[i - skew])
        while deferred and deferred[0][0] <= i:
            deferred.pop(0)[1]()
    for i in range(len(flat) - skew, len(flat)):
        emit_pv(*flat[i])
    while deferred:
        deferred.pop(0)[1]()
    if dbg is not None:
        nc.sync.dma_start(dbg[:, 0:S], qt_sb[2][:].bitcast(F32))
        nc.sync.dma_start(dbg[:, S : 2 * S], kt_sb[2][:].bitcast(F32))

    pools.close()


def make_in_maps(x_q, x_k, x_v, att_mask, W_q, b_q, W_k, b_k, W_v, b_v):
    import ml_dtypes

    f = np.float32
    f8 = ml_dtypes.float8_e4m3fn
    bf = ml_dtypes.bfloat16
    x_q, x_k, x_v = (np.asarray(a, f) for a in (x_q, x_k, x_v))
    att_mask = np.asarray(att_mask, f)
    W_q, W_k, W_v = (np.asarray(a, f) for a in (W_q, W_k, W_v))
    b_q, b_k, b_v = (np.asarray(a, f) for a in (b_q, b_k, b_v))

    def hilo(a):
        hi = a.astype(f8)
        lo = (a - hi.astype(f)).astype(f8)
        return np.ascontiguousarray(hi), np.ascontiguousarray(lo)

    in_maps = []
    for core in range(NCORES):
        b, g = divmod(core, TP)
        fsl = slice(DG * g, DG * (g + 1))
        m = {}
        for nm, x in (("xq", x_q), ("xk", x_k), ("xv", x_v)):
            hi, lo = hilo(x[b].T)
            m[nm + "h"], m[nm + "l"] = hi, lo
        for nm, W in (("wq", W_q), ("wk", W_k), ("wv", W_v)):
            hi, lo = hilo(W[fsl, :].T * WSCALE)
            m[nm + "h"], m[nm + "l"] = hi, lo
        m["bq"] = np.ascontiguousarray(
            (WSCALE * b_q[fsl]).reshape(NPAIR, 128).T.astype(f)
        )
        m["bk"] = np.ascontiguousarray(
            (WSCALE * b_k[fsl]).reshape(NPAIR, 128).T.astype(f)
        )
        m["bv"] = (WSCALE * b_v[fsl]).reshape(1, DG).astype(bf).copy()
        mk = att_mask[b, 0, 0].reshape(TCH, 128).T.astype(f)   # [128, TCH]
        m["embias"] = np.ascontiguousarray(mk - C0)
        m["sbias"] = np.ascontiguousarray(
            128.0 * LOG2E * (mk - C0) + SCHRAU_B
        )
        in_maps.append(m)
    return in_maps


def kernel(x_q, x_k, x_v, att_mask, W_q, b_q, W_k, b_k, W_v, b_v):
    global _CACHED, _CACHED_KEY
    has_bv = bool(np.any(np.asarray(b_v, np.float32) != 0.0))
    if _CACHED is None or _CACHED_KEY != has_bv:
        _CACHED = _build_core_program(has_bv)
        _CACHED_KEY = has_bv
    nc = _CACHED

    in_maps = make_in_maps(
        x_q, x_k, x_v, att_mask, W_q, b_q, W_k, b_k, W_v, b_v
    )

    global LAST_RESULTS
    trace = TRACE or os.environ.get("BASS_KERNEL_TRACE", "") == "1"
    try:
        res = run_bass_kernel_spmd(nc, in_maps, list(range(NCORES)), trace=trace)
    except Exception:
        if not trace:
            raise
        res = run_bass_kernel_spmd(nc, in_maps, list(range(NCORES)))
    LAST_RESULTS = res

    full = np.empty((B, S, H), np.float32)
    for core in range(NCORES):
        b, g = divmod(core, TP)
        r = res.results[core]["out"].reshape(S, HG, 65)
        ctx = r[..., 0:64] / r[..., 64:65] / WSCALE
        full[b, :, DG * g : DG * (g + 1)] = ctx.reshape(S, DG)
    return full
